# revision 31
# baseline (speedup 1.0000x reference)
"""Trainium2 Bass kernel for nn_Att_23313082483285 (GNN message passing).

v5 design: partition agent nodes across 8 cores (8192 each), edges routed to
the core owning hi; per core the edges split into lo/hi ctx streams (wi
halves) so ctx gather indices fit int16.

Engine-balance rework vs v3:
- GN stats via affine_mul_reduce (one fused DVE op per 128-edge chunk:
  out=x*x, accum=sum -> variance), replacing square+reduce.
- PSUM->SBUF materializations split across Act (r1T relu, y2b, g0b copies)
  and DVE (hT copy); GN applies on DVE/Pool tensor_scalar (mult+max).
- Q2 table resident in SBUF (written directly by phase 1, read by an
  SBUF-source transpose dma_gather), eliminating the Q2 DRAM round-trip;
  A1 = agts@agt_w computed inside phase 3's PSUM accumulation (no phase-1
  A1 pass, no SBUF table).
- Node-id permutation inside each 512-node group: storage row = p*4+c for
  node p + 128*c, so every [512,128] DRAM access (Q2 write, G init/read,
  residual load, out store) moves 1KB-contiguous chunks per partition
  (2x fewer effective DMA bytes than 256B-row rearranges).

GroupNorm algebra as v3: mean-centering folded into pre-centered weights
(W -> W @ (I - J/128)); per-edge/node GN is variance-only (AMR) + Sqrt +
reciprocal + tensor_scalar(mult,max); node-level norm-GN rinv cancels in the
following lin-GN and is dropped.
"""
import sys
sys.path.insert(0, '/opt/trn_rl_repo')

import numpy as np
import ml_dtypes
from contextlib import ExitStack

from concourse import bass, mybir, tile
import concourse.bacc as bacc
from concourse.bass_utils import run_bass_kernel_spmd
from concourse.masks import make_identity

bf16 = ml_dtypes.bfloat16
P = 128
N_AGT = 65536
N_CTX = 65536
E = 400000
D = 128
EPS = 1e-5
NCORES = 8
NPC = N_AGT // NCORES          # 8192 nodes per core
NGRP = NPC // (4 * P)          # 16 groups of 512 nodes
CTX_HALF = 32768
G_TILES = 16                   # tiles per gather dma op chunk (2048 idxs)
SC_TILES = 8                   # tiles per scatter-add op. HW dma_scatter_add
                               # races on duplicate indices WITHIN one op, so
                               # the host assigns each node's edges to
                               # distinct scatter chunks.
TRASH = NPC                    # scatter target row for pad edges

f32 = mybir.dt.float32
bft = mybir.dt.bfloat16
i16 = mybir.dt.int16
Act = mybir.ActivationFunctionType
Alu = mybir.AluOpType


def _snode(n):
    """Storage row for local node id n: within each 512-node group the row is
    p*4 + c for n = g*512 + c*128 + p, so phase-1/3 [P,4,D] tiles map to
    1KB-contiguous DRAM chunks per partition ((p c) f layouts)."""
    g, k = np.divmod(n, 512)
    c, p = np.divmod(k, P)
    return g * 512 + p * 4 + c


def _wrap16(flat_idx):
    """dma_gather/scatter idx layout: [16, n/16] wrapped, tiled x8."""
    w = flat_idx.reshape(-1, 16).T.astype(np.int16)
    return np.ascontiguousarray(np.tile(w, (8, 1)))


def _assign_chunks(hl, n_chunks, cap):
    """Order edges so no node appears twice in one scatter chunk.

    hl: per-edge node id. Returns edge positions array pos[len(hl)] giving
    the slot (chunk*cap + j) of each edge, or None if infeasible."""
    n = len(hl)
    order = np.argsort(hl, kind='stable')
    fill = np.zeros(n_chunks, np.int64)
    chunk_of = np.empty(n, np.int64)
    uniq, starts, counts = np.unique(hl[order], return_index=True,
                                     return_counts=True)
    node_order = np.argsort(-counts, kind='stable')
    for ni in node_order:
        k = counts[ni]
        cand = np.argpartition(fill, k - 1)[:k] if k < n_chunks else \
            np.arange(n_chunks)
        if k > n_chunks or fill[cand].max() >= cap:
            return None
        es = order[starts[ni]:starts[ni] + k]
        chunk_of[es] = cand
        fill[cand] += 1
    pos = np.empty(n, np.int64)
    nxt = np.zeros(n_chunks, np.int64)
    for e in range(n):
        c = chunk_of[e]
        pos[e] = c * cap + nxt[c]
        nxt[c] += 1
    return pos


def _host_prep(agts, ctx, agt_ctrs, ctx_ctrs, hi, wi):
    """Route edges per core (lo/hi ctx stream), build per-core arrays."""
    hi = np.asarray(hi).astype(np.int64)
    wi = np.asarray(wi).astype(np.int64)
    agts = np.asarray(agts, dtype=np.float32)
    d_all = (np.asarray(agt_ctrs, np.float32)[hi]
             - np.asarray(ctx_ctrs, np.float32)[wi])          # [E, 2]

    core = hi // NPC
    is_hi = wi >= CTX_HALF

    nlo_max = 0
    nhi_max = 0
    per_core = []
    for k in range(NCORES):
        m = core == k
        lo_e = np.nonzero(m & ~is_hi)[0]
        hi_e = np.nonzero(m & is_hi)[0]
        per_core.append((lo_e, hi_e))
        nlo_max = max(nlo_max, len(lo_e))
        nhi_max = max(nhi_max, len(hi_e))
    NLO_T = -(-(-(-nlo_max // P)) // SC_TILES) * SC_TILES  # mult of SC_TILES
    NHI_T = -(-(-(-nhi_max // P)) // SC_TILES) * SC_TILES
    cap = SC_TILES * P
    pos_all = {}
    for k in range(NCORES):
        for half, edges in enumerate(per_core[k]):
            hl = hi[edges] - k * NPC
            while True:
                n_t = NLO_T if half == 0 else NHI_T
                pos = _assign_chunks(hl, n_t // SC_TILES, cap)
                if pos is not None:
                    break
                if half == 0:
                    NLO_T += SC_TILES
                else:
                    NHI_T += SC_TILES
            pos_all[(k, half)] = pos
    NT = NLO_T + NHI_T

    cores = []
    for k in range(NCORES):
        lo_e, hi_e = per_core[k]
        dT4 = np.zeros((4, NT * P), np.float32)
        qflat = np.zeros(NT * P, np.int64)          # Q2 gather idx (pad -> 0)
        sflat = np.full(NT * P, TRASH, np.int64)    # scatter idx (pad -> trash)
        lo_flat = np.zeros(NLO_T * P, np.int64)
        hi_flat = np.zeros(NHI_T * P, np.int64)

        for half, (edges, base, flat) in enumerate(
                ((lo_e, 0, lo_flat), (hi_e, NLO_T * P, hi_flat))):
            pos = pos_all[(k, half)]
            cols = base + pos
            dT4[0, cols] = d_all[edges, 0]
            dT4[1, cols] = d_all[edges, 1]
            dT4[2, cols] = 1.0
            hl = hi[edges] - k * NPC
            qflat[cols] = hl                        # local node id (SBUF Q2)
            sflat[cols] = _snode(hl)                # permuted Gd storage rows
            flat[pos] = wi[edges] if half == 0 else wi[edges] - CTX_HALF

        # resb rows permuted to storage order (resb[_snode(n)] = agts[n]) so
        # the phase-3 "(p c) f" load yields tile[p,c] = node c*128+p, matching
        # the Gd tiles. kernel() un-permutes the output with the same map.
        ag_k = agts[k * NPC:(k + 1) * NPC]
        resb = np.empty_like(ag_k)
        resb[_snode(np.arange(NPC))] = ag_k
        cores.append(dict(
            agtsT=np.ascontiguousarray(ag_k.T.astype(bf16)),
            resb=np.ascontiguousarray(resb.astype(bf16)),
            dT4=dT4.astype(bf16),
            qidx=_wrap16(qflat),
            sidx=_wrap16(sflat),
            widx_lo=_wrap16(lo_flat),
            widx_hi=_wrap16(hi_flat),
        ))
    return cores, NLO_T, NHI_T


def _build_program(NLO_T, NHI_T):
    NT = NLO_T + NHI_T
    NST = NT // 4

    nc = bacc.Bacc("TRN2", target_bir_lowering=False, debug=False,
                   enable_asserts=False, num_devices=NCORES,
                   dynamic_dma_scratch_size=16384)

    def din(name, shape, dt):
        return nc.dram_tensor(name, list(shape), dt, kind="ExternalInput").ap()

    t_agtsT = din("agtsT", (P, NPC), bft)
    t_resb = din("resb", (NPC, D), bft)
    t_ctx = din("ctx_bf", (N_CTX, D), bft)
    t_dT4 = din("dT4", (4, NT * P), bft)
    t_qidx = din("qidx", (P, NT * P // 16), i16)
    t_sidx = din("sidx", (P, NT * P // 16), i16)
    t_wlo = din("widx_lo", (P, NLO_T * P // 16), i16)
    t_whi = din("widx_hi", (P, NHI_T * P // 16), i16)
    wnames = ["w1_aug", "W2c", "Wdc", "Wcc", "Wqc", "qwc",
              "agtwc", "ctxw2c", "linwc"]
    t_w = {n: din(n, (4, D) if n == "w1_aug" else (D, D), bft) for n in wnames}
    t_out = nc.dram_tensor("out", [NPC, D], bft, kind="ExternalOutput").ap()

    with tile.TileContext(nc) as tc, ExitStack() as ctx:
        const = ctx.enter_context(tc.tile_pool(name="const", bufs=1))
        big = ctx.enter_context(tc.tile_pool(name="big", bufs=1))
        dram = ctx.enter_context(tc.tile_pool(name="dram", bufs=1, space="DRAM"))
        sb = ctx.enter_context(tc.tile_pool(name="sb", bufs=4))
        gb = ctx.enter_context(tc.tile_pool(name="gb", bufs=3))
        gp = ctx.enter_context(tc.tile_pool(name="gp", bufs=4))
        psY = ctx.enter_context(tc.tile_pool(name="psY", bufs=2, space="PSUM"))
        psM = ctx.enter_context(tc.tile_pool(name="psM", bufs=2, space="PSUM"))
        psK = ctx.enter_context(tc.tile_pool(name="psK", bufs=2, space="PSUM"))
        psL = ctx.enter_context(tc.tile_pool(name="psL", bufs=2, space="PSUM"))

        # ---------- constants ----------
        ident = const.tile([P, P], f32)
        make_identity(nc, ident[:])
        ident_bf = const.tile([P, P], bft)
        nc.vector.tensor_copy(ident_bf[:], ident[:])
        eps_t = const.tile([P, 1], f32)
        nc.gpsimd.memset(eps_t[:], EPS)
        zt = const.tile([P, 4, D], bft)
        nc.gpsimd.memset(zt[:], 0.0)
        w_sb = {}
        for n in wnames:
            shp = [4, D] if n == "w1_aug" else [D, D]
            w_sb[n] = const.tile(shp, bft, name=f"w_{n}")
            nc.sync.dma_start(w_sb[n][:], t_w[n][:])

        # big resident tensors
        agtsT = big.tile([P, NPC], bft)
        nc.sync.dma_start(agtsT[:], t_agtsT[:])
        qidx = big.tile([P, NT * P // 16], i16)
        nc.sync.dma_start(qidx[:], t_qidx[:])
        sidx = big.tile([P, NT * P // 16], i16)
        nc.sync.dma_start(sidx[:], t_sidx[:])
        wlo = big.tile([P, NLO_T * P // 16], i16)
        nc.sync.dma_start(wlo[:], t_wlo[:])
        whi = big.tile([P, NHI_T * P // 16], i16)
        nc.sync.dma_start(whi[:], t_whi[:])
        # Q2 node table resident in SBUF: token n -> partition n%128,
        # 256B rank-stripe n//128 (SBUF-source dma_gather layout).
        Q2sb = big.tile([P, NPC // P, D], bft)

        Gd = dram.tile([NPC + P, D], bft)

        def gn_stats(src_b, tag):
            """variance (pre-centered input) via affine_mul_reduce per chunk:
            scr = src*src, ss[:, c] = sum(scr). Returns rinv [P,4] f32 and
            sd [P,4] f32 (sqrt(var+eps))."""
            scr = sb.tile([P, D], bft, tag=f"scr{tag}")
            ss = sb.tile([P, 4], f32, tag=f"ss{tag}")
            for c in range(4):
                nc.vector.affine_mul_reduce(scr[:], ss[:, c:c + 1],
                                            src_b[:, c, :], src_b[:, c, :],
                                            1.0, 0.0)
            sd = sb.tile([P, 4], f32, tag=f"sd{tag}")
            nc.scalar.activation(sd[:], ss[:], Act.Sqrt, bias=eps_t[:],
                                 scale=1.0 / D)
            rinv = sb.tile([P, 4], f32, tag=f"ri{tag}")
            nc.vector.reciprocal(rinv[:], sd[:])
            return rinv, sd

        # ---------- phase 1: node-level precompute ----------
        # Q2 = (r_q*relu(agts@q_w')) @ Wq' -> Q2sb SBUF table
        def p1_stages(g):
            ps_q = psL.tile([P, 4, D], f32, space="PSUM", tag="l")
            for c in range(4):
                j = g * 4 + c
                nc.tensor.matmul(ps_q[:, c, :], lhsT=agtsT[:, j * P:(j + 1) * P],
                                 rhs=w_sb["qwc"][:], start=True, stop=True)
            yield
            yqb = sb.tile([P, 4, D], bft, tag="yqb")
            nc.scalar.copy(yqb[:], ps_q[:])
            yield
            r_q, _ = gn_stats(yqb, "q")
            yield
            qn = sb.tile([P, 4, D], bft, tag="qn")
            for c in range(4):
                nc.vector.tensor_scalar(qn[:, c, :], yqb[:, c, :],
                                        r_q[:, c:c + 1], 0.0,
                                        op0=Alu.mult, op1=Alu.max)
            yield
            ps_t = psK.tile([P, 4, D], bft, space="PSUM", tag="k")
            for c in range(4):
                nc.tensor.transpose(ps_t[:, c, :], qn[:, c, :], ident_bf[:])
            yield
            qnT = sb.tile([P, 4, D], bft, tag="qnT")
            nc.vector.tensor_copy(qnT[:], ps_t[:])
            yield
            ps_q2 = psM.tile([P, 4, D], f32, space="PSUM", tag="m")
            for c in range(4):
                nc.tensor.matmul(ps_q2[:, c, :], lhsT=qnT[:, c, :],
                                 rhs=w_sb["Wqc"][:], start=True, stop=True)
            yield
            # ps_q2[p, c, :] = Q2[node g*512+c*128+p] -> Q2sb rank g*4+c ok
            nc.scalar.copy(Q2sb[:, g * 4:(g + 1) * 4, :], ps_q2[:])
            nc.sync.dma_start(
                Gd[g * 512:(g + 1) * 512, :].rearrange(
                    "(p c) f -> p c f", p=P),
                zt[:])
            yield

        def run_interleaved(mk, n, ilv):
            i = 0
            while i < n:
                m = min(ilv, n - i)
                gens = [mk(i + j) for j in range(m)]
                alive = True
                while alive:
                    alive = False
                    for g_ in gens:
                        try:
                            next(g_)
                            alive = True
                        except StopIteration:
                            pass
                i += m

        # ---------- phase 2: edge pipeline ----------
        dt_bufs = {}
        q2_bufs = {}
        q2_enabled = [False]
        lo_bufs = {}
        hi_bufs = {}
        gst_bufs = {}

        def issue_dt(gi):
            nt = min(G_TILES, NT - gi * G_TILES)
            buf = gb.tile([4, G_TILES * P], bft, tag="dt4")
            nc.sync.dma_start(buf[:, :nt * P],
                              t_dT4[:, gi * G_TILES * P:(gi * G_TILES + nt) * P])
            dt_bufs[gi] = buf

        def issue_q2(gi):
            nt = min(G_TILES, NT - gi * G_TILES)
            buf = gb.tile([P, 1, G_TILES * P], bft, tag="q2g")
            nc.gpsimd.dma_gather(
                out_ap=buf[:, :, :nt * P], in_ap=Q2sb[:],
                idxs_ap=qidx[:, gi * G_TILES * 8:(gi * G_TILES + nt) * 8],
                num_idxs=nt * P, num_idxs_reg=nt * P, elem_size=D,
                transpose=True, single_packet=False,
                sbuf_tokens_per_rank=P, sbuf_free_dim_per_rank=D * 2)
            q2_bufs[gi] = buf

        def issue_w(gi, half):
            n_str, src, idxt, bufs, tag = (
                (NLO_T, t_ctx[:CTX_HALF, :], wlo, lo_bufs, "clo") if half == 0
                else (NHI_T, t_ctx[CTX_HALF:, :], whi, hi_bufs, "chi"))
            nt = min(G_TILES, n_str - gi * G_TILES)
            buf = gb.tile([P, 1, G_TILES * P], bft, tag=tag)
            nc.gpsimd.dma_gather(
                out_ap=buf[:, :, :nt * P], in_ap=src,
                idxs_ap=idxt[:, gi * G_TILES * 8:(gi * G_TILES + nt) * 8],
                num_idxs=nt * P, num_idxs_reg=nt * P, elem_size=D,
                transpose=True, single_packet=False)
            bufs[gi] = buf

        def tmeta(t):
            return (0, t) if t < NLO_T else (1, t - NLO_T)

        def ensure_chunk(gi):
            """Issue gather DMAs covering global gather-chunk gi."""
            if gi * G_TILES >= NT:
                return
            if gi not in dt_bufs:
                issue_dt(gi)
            for t in range(gi * G_TILES, min((gi + 1) * G_TILES, NT)):
                half, si = tmeta(t)
                wgi = si // G_TILES
                if half == 0 and wgi not in lo_bufs:
                    issue_w(wgi, 0)
                if half == 1 and wgi not in hi_bufs:
                    issue_w(wgi, 1)
            if q2_enabled[0] and gi not in q2_bufs:
                issue_q2(gi)

        # prefetch the first two gather chunks' dT4 + ctx gathers before
        # phase 1 so the DMA engines fill the pipeline under phase-1 compute
        # (q2 gathers depend on the Q2sb table and must stay after phase 1).
        for _gi in range(3):
            if _gi * G_TILES < NT:
                issue_dt(_gi)
                for _t in range(_gi * G_TILES, min((_gi + 1) * G_TILES, NT)):
                    _half, _si = tmeta(_t)
                    _wgi = _si // G_TILES
                    if _half == 0 and _wgi not in lo_bufs:
                        issue_w(_wgi, 0)
                    if _half == 1 and _wgi not in hi_bufs:
                        issue_w(_wgi, 1)

        # (phase-2 gather helpers defined above, before phase 1)

        def st_stages(s):
            """Pipeline for super-tile s (4 tiles), stage-interleaved."""
            tiles = [4 * s + c for c in range(4)]
            gi0, off0 = divmod(4 * s, G_TILES)
            sc0, soff0 = divmod(4 * s, SC_TILES)
            ensure_chunk(gi0)
            ensure_chunk(gi0 + 1)
            if sc0 not in gst_bufs:
                gst_bufs[sc0] = gp.tile([P, SC_TILES, D], bft, tag="gst",
                                        name="gst")
            yield
            # L1: y1T [f, 512] = w1_aug.T @ dT4
            ps_y1 = psY.tile([P, 4 * D], f32, space="PSUM", tag="y")
            nc.tensor.matmul(ps_y1[:], lhsT=w_sb["w1_aug"][:],
                             rhs=dt_bufs[gi0][:, off0 * P:(off0 + 4) * P],
                             start=True, stop=True)
            yield
            # relu -> r1T bf16 (Act)
            r1T = sb.tile([P, 4 * D], bft, tag="r1T")
            nc.scalar.activation(r1T[:], ps_y1[:], Act.Relu)
            yield
            # L2 (centered) -> ps2 edge-major
            ps2 = psM.tile([P, 4, D], f32, space="PSUM", tag="m")
            for c in range(4):
                nc.tensor.matmul(ps2[:, c, :], lhsT=r1T[:, c * D:(c + 1) * D],
                                 rhs=w_sb["W2c"][:], start=True, stop=True)
            yield
            # y2b copy (Act)
            y2b = sb.tile([P, 4, D], bft, tag="y2b")
            nc.scalar.copy(y2b[:], ps2[:])
            yield
            # GN1 stats (DVE AMR + Act sqrt + DVE recip)
            r1e, _ = gn_stats(y2b, "d")
            yield
            # GN1 apply -> h (DVE)
            h = sb.tile([P, 4, D], bft, tag="h")
            for c in range(4):
                nc.vector.tensor_scalar(h[:, c, :], y2b[:, c, :],
                                        r1e[:, c:c + 1], 0.0,
                                        op0=Alu.mult, op1=Alu.max)
            yield
            # T(h)
            psT = psK.tile([P, 4, D], bft, space="PSUM", tag="k")
            for c in range(4):
                nc.tensor.transpose(psT[:, c, :], h[:, c, :], ident_bf[:])
            yield
            # hT copy split Act/DVE for engine balance
            hT = sb.tile([P, 4, D], bft, tag="hT")
            nc.scalar.copy(hT[:, 0:2, :], psT[:, 0:2, :])
            nc.vector.tensor_copy(hT[:, 2:4, :], psT[:, 2:4, :])
            yield
            # C1 = h@Wd' + ctx[wi]@Wc' + Q2[hi] (identity mm)
            ps3 = psL.tile([P, 4, D], f32, space="PSUM", tag="l")
            for c, t in enumerate(tiles):
                half, si = tmeta(t)
                gi, off = divmod(si, G_TILES)
                cbuf = lo_bufs[gi] if half == 0 else hi_bufs[gi]
                qgi, qoff = divmod(t, G_TILES)
                nc.tensor.matmul(ps3[:, c, :], lhsT=hT[:, c, :],
                                 rhs=w_sb["Wdc"][:], start=True, stop=False)
                nc.tensor.matmul(ps3[:, c, :],
                                 lhsT=cbuf[:, 0, off * P:(off + 1) * P],
                                 rhs=w_sb["Wcc"][:], start=False, stop=False)
                nc.tensor.matmul(ps3[:, c, :],
                                 lhsT=q2_bufs[qgi][:, 0, qoff * P:(qoff + 1) * P],
                                 rhs=ident_bf[:],
                                 start=False, stop=True)
            yield
            # g0b copy (Act)
            g0b = sb.tile([P, 4, D], bft, tag="g0b")
            nc.scalar.copy(g0b[:], ps3[:])
            yield
            # GN2 stats
            r2, _ = gn_stats(g0b, "c")
            yield
            # GN2 apply -> g into scatter staging (DVE)
            gst = gst_bufs[sc0]
            for c in range(4):
                nc.vector.tensor_scalar(gst[:, soff0 + c, :], g0b[:, c, :],
                                        r2[:, c:c + 1], 0.0,
                                        op0=Alu.mult, op1=Alu.max)
            yield
            # scatter-add chunk when staging is full
            if soff0 + 4 == SC_TILES:
                nc.gpsimd.dma_scatter_add(
                    out_ap=Gd[:], in_ap=gst[:, :, :],
                    idxs_ap=sidx[:, sc0 * SC_TILES * 8:(sc0 + 1) * SC_TILES * 8],
                    num_idxs=SC_TILES * P, num_idxs_reg=SC_TILES * P,
                    elem_size=D, single_packet=False)
            yield

        # pad-row zero for scatter TRASH targets: no deps, emit before the
        # merged pool so early scatters don't order against it awkwardly.
        nc.sync.dma_start(Gd[NPC:NPC + P, :], zt[:, 0, :])

        # Merged emission: phase-1 groups + the first EARLY phase-2
        # supertiles. Their stages up to hT depend only on the prefetched
        # dT4/ctx chunks, so they soak up phase-1's idle engine time; their
        # q2-dependent stages simply wait (the q2 descgen head-blocking the
        # Pool queue is harmless here — no other Pool work is pending until
        # after phase 1).
        EARLY = 6
        PRE_Q2_STEPS = 9            # stages 0..8: through the hT copy
        p1g = [p1_stages(g) for g in range(NGRP)]
        eg = [st_stages(s) for s in range(EARLY)]
        done1 = [False] * len(p1g)
        ecnt = [0] * EARLY
        while not all(done1):
            for i, g_ in enumerate(p1g):
                if not done1[i]:
                    try:
                        next(g_)
                    except StopIteration:
                        done1[i] = True
            for j, g_ in enumerate(eg):
                if ecnt[j] < PRE_Q2_STEPS:
                    try:
                        next(g_)
                        ecnt[j] += 1
                    except StopIteration:
                        ecnt[j] = PRE_Q2_STEPS
        # phase 1 fully emitted: q2 gathers may now be issued (they must be
        # emitted after every Q2sb write for correct dependency tracking)
        q2_enabled[0] = True
        ensure_chunk(0)
        ensure_chunk(1)
        alive = True
        while alive:
            alive = False
            for g_ in eg:
                try:
                    next(g_)
                    alive = True
                except StopIteration:
                    pass


        ILV = 8
        grp = 0
        while EARLY + grp * ILV < NST:
            n = min(ILV, NST - EARLY - grp * ILV)
            gens = [st_stages(EARLY + grp * ILV + j) for j in range(n)]
            alive = True
            while alive:
                alive = False
                for g_ in gens:
                    try:
                        next(g_)
                        alive = True
                    except StopIteration:
                        pass
            grp += 1

        # ---------- phase 3: node finale ----------
        # Phase-3 "(p c) f" loads of the permuted Gd/resb rows yield
        # tile[p,c] = node g*512 + c*128 + p — the original mapping, so the
        # A1 term can be computed directly from the matching agtsT columns.
        def p3_stages(g):
            # res load first: it has no scatter dependency, so it streams
            # during the end-of-phase-2 drain instead of queueing behind the
            # scatter-gated Gsb load in SP's in-order queue.
            res_sb = sb.tile([P, 4, D], bft, tag="res_sb")
            nc.sync.dma_start(
                res_sb[:],
                t_resb[g * 512:(g + 1) * 512, :].rearrange(
                    "(p c) f -> p c f", p=P))
            Gsb = sb.tile([P, 4, D], bft, tag="Gsb")
            nc.sync.dma_start(
                Gsb[:],
                Gd[g * 512:(g + 1) * 512, :].rearrange("(p c) f -> p c f", p=P))
            yield
            psTG = psK.tile([P, 4, D], bft, space="PSUM", tag="k")
            for c in range(4):
                nc.tensor.transpose(psTG[:, c, :], Gsb[:, c, :], ident_bf[:])
            yield
            GT = sb.tile([P, 4, D], bft, tag="GT")
            nc.vector.tensor_copy(GT[:], psTG[:])
            yield
            # S^T = ctx_w2'^T G^T + A1^T (feature-major)
            ps_S = psM.tile([P, 4, D], f32, space="PSUM", tag="m")
            for c in range(4):
                nc.tensor.matmul(ps_S[:, c, :], lhsT=w_sb["ctxw2c"][:],
                                 rhs=GT[:, c, :], start=True, stop=False)
                j = g * 4 + c
                nc.tensor.matmul(ps_S[:, c, :], lhsT=w_sb["agtwc"][:],
                                 rhs=agtsT[:, j * P:(j + 1) * P],
                                 start=False, stop=True)
            yield
            o1uT = sb.tile([P, 4, D], bft, tag="o1uT")
            nc.scalar.activation(o1uT[:], ps_S[:], Act.Relu)
            yield
            ps_l = psL.tile([P, 4, D], f32, space="PSUM", tag="l")
            for c in range(4):
                nc.tensor.matmul(ps_l[:, c, :], lhsT=o1uT[:, c, :],
                                 rhs=w_sb["linwc"][:], start=True, stop=True)
            yield
            zb = sb.tile([P, 4, D], bft, tag="zb")
            nc.scalar.copy(zb[:], ps_l[:])
            yield
            r_l, _ = gn_stats(zb, "l")
            yield
            # fin = relu(zb*rinv + res): STT per chunk then one max
            fin = sb.tile([P, 4, D], bft, tag="fin")
            for c in range(4):
                nc.vector.scalar_tensor_tensor(
                    out=fin[:, c, :], in0=zb[:, c, :],
                    scalar=r_l[:, c:c + 1], in1=res_sb[:, c, :],
                    op0=Alu.mult, op1=Alu.add)
            yield
            nc.gpsimd.tensor_scalar_max(fin[:], fin[:], 0.0)
            nc.sync.dma_start(
                t_out[g * 512:(g + 1) * 512, :].rearrange(
                    "(p c) f -> p c f", p=P),
                fin[:])
            yield

        run_interleaved(p3_stages, NGRP, 8)

    nc.compile()
    return nc


_cached = {}
_extra_run_kwargs = {}
_last_results = None


def run_traced(inputs):
    global _extra_run_kwargs
    _extra_run_kwargs = dict(trace=True)
    try:
        kernel(**inputs)
    finally:
        _extra_run_kwargs = {}
    return _last_results


def kernel(agts, ctx, agt_ctrs, ctx_ctrs, hi, wi,
           dist_w1, dist_b1, dist_w2, dist_gw, dist_gb,
           q_w, q_gw, q_gb,
           ctx_w1, ctx_gw, ctx_gb, ctx_w2,
           agt_w, norm_w, norm_b,
           lin_w, lin_gw, lin_gb):
    for name, arr, val in (("dist_gw", dist_gw, 1), ("dist_gb", dist_gb, 0),
                           ("q_gw", q_gw, 1), ("q_gb", q_gb, 0),
                           ("ctx_gw", ctx_gw, 1), ("ctx_gb", ctx_gb, 0),
                           ("norm_w", norm_w, 1), ("norm_b", norm_b, 0),
                           ("lin_gw", lin_gw, 1), ("lin_gb", lin_gb, 0)):
        assert np.allclose(np.asarray(arr), val), f"{name} must be trivial"

    C = np.eye(D, dtype=np.float64) - 1.0 / D   # GN mean-centering projector
    ctx_w1 = np.asarray(ctx_w1, np.float64)
    w1 = np.asarray(dist_w1, np.float32)
    b1 = np.asarray(dist_b1, np.float32)
    w1_aug = np.zeros((4, D), np.float32)
    w1_aug[0:2] = w1
    w1_aug[2] = b1
    weights = dict(
        w1_aug=w1_aug.astype(bf16),
        W2c=(np.asarray(dist_w2, np.float64) @ C).astype(bf16),
        Wdc=(ctx_w1[0:D] @ C).astype(bf16),
        Wqc=(ctx_w1[D:2 * D] @ C).astype(bf16),
        Wcc=(ctx_w1[2 * D:3 * D] @ C).astype(bf16),
        qwc=(np.asarray(q_w, np.float64) @ C).astype(bf16),
        agtwc=(np.asarray(agt_w, np.float64) @ C).astype(bf16),
        ctxw2c=(np.asarray(ctx_w2, np.float64) @ C).astype(bf16),
        linwc=(np.asarray(lin_w, np.float64) @ C).astype(bf16),
    )

    cores, NLO_T, NHI_T = _host_prep(agts, ctx, agt_ctrs, ctx_ctrs, hi, wi)
    key = (NLO_T, NHI_T)
    if key not in _cached:
        _cached[key] = _build_program(NLO_T, NHI_T)
    nc = _cached[key]

    shared = dict(ctx_bf=np.ascontiguousarray(
        np.asarray(ctx, np.float32).astype(bf16)), **weights)
    in_maps = []
    for k in range(NCORES):
        m = dict(cores[k])
        m.update(shared)
        in_maps.append(m)

    res = run_bass_kernel_spmd(nc, in_maps, core_ids=list(range(NCORES)),
                               **_extra_run_kwargs)
    globals()["_last_results"] = res
    sperm = _snode(np.arange(NPC))   # out rows are in storage order
    out = np.concatenate([res.results[k]["out"][sperm] for k in range(NCORES)],
                         axis=0)
    return out.astype(np.float32)


if __name__ == "__main__":
    pass


# revision 32
# speedup vs baseline: 1.0128x; 1.0128x over previous
"""Trainium2 Bass kernel for nn_Att_23313082483285 (GNN message passing).

v5 design: partition agent nodes across 8 cores (8192 each), edges routed to
the core owning hi; per core the edges split into lo/hi ctx streams (wi
halves) so ctx gather indices fit int16.

Engine-balance rework vs v3:
- GN stats via affine_mul_reduce (one fused DVE op per 128-edge chunk:
  out=x*x, accum=sum -> variance), replacing square+reduce.
- PSUM->SBUF materializations split across Act (r1T relu, y2b, g0b copies)
  and DVE (hT copy); GN applies on DVE/Pool tensor_scalar (mult+max).
- Q2 table resident in SBUF (written directly by phase 1, read by an
  SBUF-source transpose dma_gather), eliminating the Q2 DRAM round-trip;
  A1 = agts@agt_w computed inside phase 3's PSUM accumulation (no phase-1
  A1 pass, no SBUF table).
- Node-id permutation inside each 512-node group: storage row = p*4+c for
  node p + 128*c, so every [512,128] DRAM access (Q2 write, G init/read,
  residual load, out store) moves 1KB-contiguous chunks per partition
  (2x fewer effective DMA bytes than 256B-row rearranges).

GroupNorm algebra as v3: mean-centering folded into pre-centered weights
(W -> W @ (I - J/128)); per-edge/node GN is variance-only (AMR) + Sqrt +
reciprocal + tensor_scalar(mult,max); node-level norm-GN rinv cancels in the
following lin-GN and is dropped.
"""
import sys
sys.path.insert(0, '/opt/trn_rl_repo')

import numpy as np
import ml_dtypes
from contextlib import ExitStack

from concourse import bass, mybir, tile
import concourse.bacc as bacc
from concourse.bass_utils import run_bass_kernel_spmd
from concourse.masks import make_identity

bf16 = ml_dtypes.bfloat16
P = 128
N_AGT = 65536
N_CTX = 65536
E = 400000
D = 128
EPS = 1e-5
NCORES = 8
NPC = N_AGT // NCORES          # 8192 nodes per core
NGRP = NPC // (4 * P)          # 16 groups of 512 nodes
CTX_HALF = 32768
G_TILES = 16                   # tiles per gather dma op chunk (2048 idxs)
SC_TILES = 8                   # tiles per scatter-add op. HW dma_scatter_add
                               # races on duplicate indices WITHIN one op, so
                               # the host assigns each node's edges to
                               # distinct scatter chunks.
TRASH = NPC                    # scatter target row for pad edges

f32 = mybir.dt.float32
bft = mybir.dt.bfloat16
i16 = mybir.dt.int16
Act = mybir.ActivationFunctionType
Alu = mybir.AluOpType


def _snode(n):
    """Storage row for local node id n: within each 512-node group the row is
    p*4 + c for n = g*512 + c*128 + p, so phase-1/3 [P,4,D] tiles map to
    1KB-contiguous DRAM chunks per partition ((p c) f layouts)."""
    g, k = np.divmod(n, 512)
    c, p = np.divmod(k, P)
    return g * 512 + p * 4 + c


def _wrap16(flat_idx):
    """dma_gather/scatter idx layout: [16, n/16] wrapped, tiled x8."""
    w = flat_idx.reshape(-1, 16).T.astype(np.int16)
    return np.ascontiguousarray(np.tile(w, (8, 1)))


def _assign_chunks(hl, n_chunks, cap):
    """Order edges so no node appears twice in one scatter chunk.

    hl: per-edge node id. Returns edge positions array pos[len(hl)] giving
    the slot (chunk*cap + j) of each edge, or None if infeasible."""
    n = len(hl)
    order = np.argsort(hl, kind='stable')
    fill = np.zeros(n_chunks, np.int64)
    chunk_of = np.empty(n, np.int64)
    uniq, starts, counts = np.unique(hl[order], return_index=True,
                                     return_counts=True)
    node_order = np.argsort(-counts, kind='stable')
    for ni in node_order:
        k = counts[ni]
        cand = np.argpartition(fill, k - 1)[:k] if k < n_chunks else \
            np.arange(n_chunks)
        if k > n_chunks or fill[cand].max() >= cap:
            return None
        es = order[starts[ni]:starts[ni] + k]
        chunk_of[es] = cand
        fill[cand] += 1
    pos = np.empty(n, np.int64)
    nxt = np.zeros(n_chunks, np.int64)
    for e in range(n):
        c = chunk_of[e]
        pos[e] = c * cap + nxt[c]
        nxt[c] += 1
    return pos


def _host_prep(agts, ctx, agt_ctrs, ctx_ctrs, hi, wi):
    """Route edges per core (lo/hi ctx stream), build per-core arrays."""
    hi = np.asarray(hi).astype(np.int64)
    wi = np.asarray(wi).astype(np.int64)
    agts = np.asarray(agts, dtype=np.float32)
    d_all = (np.asarray(agt_ctrs, np.float32)[hi]
             - np.asarray(ctx_ctrs, np.float32)[wi])          # [E, 2]

    core = hi // NPC
    is_hi = wi >= CTX_HALF

    nlo_max = 0
    nhi_max = 0
    per_core = []
    for k in range(NCORES):
        m = core == k
        lo_e = np.nonzero(m & ~is_hi)[0]
        hi_e = np.nonzero(m & is_hi)[0]
        per_core.append((lo_e, hi_e))
        nlo_max = max(nlo_max, len(lo_e))
        nhi_max = max(nhi_max, len(hi_e))
    NLO_T = -(-(-(-nlo_max // P)) // SC_TILES) * SC_TILES  # mult of SC_TILES
    NHI_T = -(-(-(-nhi_max // P)) // SC_TILES) * SC_TILES
    cap = SC_TILES * P
    pos_all = {}
    for k in range(NCORES):
        for half, edges in enumerate(per_core[k]):
            hl = hi[edges] - k * NPC
            while True:
                n_t = NLO_T if half == 0 else NHI_T
                pos = _assign_chunks(hl, n_t // SC_TILES, cap)
                if pos is not None:
                    break
                if half == 0:
                    NLO_T += SC_TILES
                else:
                    NHI_T += SC_TILES
            pos_all[(k, half)] = pos
    NT = NLO_T + NHI_T

    cores = []
    for k in range(NCORES):
        lo_e, hi_e = per_core[k]
        dT4 = np.zeros((4, NT * P), np.float32)
        qflat = np.zeros(NT * P, np.int64)          # Q2 gather idx (pad -> 0)
        sflat = np.full(NT * P, TRASH, np.int64)    # scatter idx (pad -> trash)
        lo_flat = np.zeros(NLO_T * P, np.int64)
        hi_flat = np.zeros(NHI_T * P, np.int64)

        for half, (edges, base, flat) in enumerate(
                ((lo_e, 0, lo_flat), (hi_e, NLO_T * P, hi_flat))):
            pos = pos_all[(k, half)]
            cols = base + pos
            dT4[0, cols] = d_all[edges, 0]
            dT4[1, cols] = d_all[edges, 1]
            dT4[2, cols] = 1.0
            hl = hi[edges] - k * NPC
            qflat[cols] = hl                        # local node id (SBUF Q2)
            sflat[cols] = _snode(hl)                # permuted Gd storage rows
            flat[pos] = wi[edges] if half == 0 else wi[edges] - CTX_HALF

        # resb rows permuted to storage order (resb[_snode(n)] = agts[n]) so
        # the phase-3 "(p c) f" load yields tile[p,c] = node c*128+p, matching
        # the Gd tiles. kernel() un-permutes the output with the same map.
        ag_k = agts[k * NPC:(k + 1) * NPC]
        resb = np.empty_like(ag_k)
        resb[_snode(np.arange(NPC))] = ag_k
        cores.append(dict(
            agtsT=np.ascontiguousarray(ag_k.T.astype(bf16)),
            resb=np.ascontiguousarray(resb.astype(bf16)),
            dT4=dT4.astype(bf16),
            qidx=_wrap16(qflat),
            sidx=_wrap16(sflat),
            widx_lo=_wrap16(lo_flat),
            widx_hi=_wrap16(hi_flat),
        ))
    return cores, NLO_T, NHI_T


def _build_program(NLO_T, NHI_T):
    NT = NLO_T + NHI_T
    NST = NT // 4

    nc = bacc.Bacc("TRN2", target_bir_lowering=False, debug=False,
                   enable_asserts=False, num_devices=NCORES,
                   dynamic_dma_scratch_size=16384)

    def din(name, shape, dt):
        return nc.dram_tensor(name, list(shape), dt, kind="ExternalInput").ap()

    t_agtsT = din("agtsT", (P, NPC), bft)
    t_resb = din("resb", (NPC, D), bft)
    t_ctx = din("ctx_bf", (N_CTX, D), bft)
    t_dT4 = din("dT4", (4, NT * P), bft)
    t_qidx = din("qidx", (P, NT * P // 16), i16)
    t_sidx = din("sidx", (P, NT * P // 16), i16)
    t_wlo = din("widx_lo", (P, NLO_T * P // 16), i16)
    t_whi = din("widx_hi", (P, NHI_T * P // 16), i16)
    wnames = ["w1_aug", "W2c", "Wdc", "Wcc", "Wqc", "qwc",
              "agtwc", "ctxw2c", "linwc"]
    t_w = {n: din(n, (4, D) if n == "w1_aug" else (D, D), bft) for n in wnames}
    t_out = nc.dram_tensor("out", [NPC, D], bft, kind="ExternalOutput").ap()

    with tile.TileContext(nc) as tc, ExitStack() as ctx:
        const = ctx.enter_context(tc.tile_pool(name="const", bufs=1))
        big = ctx.enter_context(tc.tile_pool(name="big", bufs=1))
        dram = ctx.enter_context(tc.tile_pool(name="dram", bufs=1, space="DRAM"))
        sb = ctx.enter_context(tc.tile_pool(name="sb", bufs=4))
        gb = ctx.enter_context(tc.tile_pool(name="gb", bufs=3))
        gp = ctx.enter_context(tc.tile_pool(name="gp", bufs=4))
        psY = ctx.enter_context(tc.tile_pool(name="psY", bufs=2, space="PSUM"))
        psM = ctx.enter_context(tc.tile_pool(name="psM", bufs=2, space="PSUM"))
        psK = ctx.enter_context(tc.tile_pool(name="psK", bufs=2, space="PSUM"))
        psL = ctx.enter_context(tc.tile_pool(name="psL", bufs=2, space="PSUM"))

        # ---------- constants ----------
        ident = const.tile([P, P], f32)
        make_identity(nc, ident[:])
        ident_bf = const.tile([P, P], bft)
        nc.vector.tensor_copy(ident_bf[:], ident[:])
        eps_t = const.tile([P, 1], f32)
        nc.gpsimd.memset(eps_t[:], EPS)
        zt = const.tile([P, 4, D], bft)
        nc.gpsimd.memset(zt[:], 0.0)
        w_sb = {}
        for n in wnames:
            shp = [4, D] if n == "w1_aug" else [D, D]
            w_sb[n] = const.tile(shp, bft, name=f"w_{n}")
            nc.sync.dma_start(w_sb[n][:], t_w[n][:])

        # big resident tensors
        agtsT = big.tile([P, NPC], bft)
        nc.sync.dma_start(agtsT[:], t_agtsT[:])
        qidx = big.tile([P, NT * P // 16], i16)
        nc.sync.dma_start(qidx[:], t_qidx[:])
        sidx = big.tile([P, NT * P // 16], i16)
        nc.sync.dma_start(sidx[:], t_sidx[:])
        wlo = big.tile([P, NLO_T * P // 16], i16)
        nc.sync.dma_start(wlo[:], t_wlo[:])
        whi = big.tile([P, NHI_T * P // 16], i16)
        nc.sync.dma_start(whi[:], t_whi[:])
        # Q2 node table resident in SBUF: token n -> partition n%128,
        # 256B rank-stripe n//128 (SBUF-source dma_gather layout).
        Q2sb = big.tile([P, NPC // P, D], bft)

        Gd = dram.tile([NPC + P, D], bft)

        def gn_stats(src_b, tag):
            """variance (pre-centered input) via affine_mul_reduce per chunk:
            scr = src*src, ss[:, c] = sum(scr). Returns rinv [P,4] f32 and
            sd [P,4] f32 (sqrt(var+eps))."""
            scr = sb.tile([P, D], bft, tag=f"scr{tag}")
            ss = sb.tile([P, 4], f32, tag=f"ss{tag}")
            for c in range(4):
                nc.vector.affine_mul_reduce(scr[:], ss[:, c:c + 1],
                                            src_b[:, c, :], src_b[:, c, :],
                                            1.0, 0.0)
            sd = sb.tile([P, 4], f32, tag=f"sd{tag}")
            nc.scalar.activation(sd[:], ss[:], Act.Sqrt, bias=eps_t[:],
                                 scale=1.0 / D)
            rinv = sb.tile([P, 4], f32, tag=f"ri{tag}")
            nc.vector.reciprocal(rinv[:], sd[:])
            return rinv, sd

        # ---------- phase 1: node-level precompute ----------
        # Q2 = (r_q*relu(agts@q_w')) @ Wq' -> Q2sb SBUF table
        def p1_stages(g):
            ps_q = psL.tile([P, 4, D], f32, space="PSUM", tag="l")
            for c in range(4):
                j = g * 4 + c
                nc.tensor.matmul(ps_q[:, c, :], lhsT=agtsT[:, j * P:(j + 1) * P],
                                 rhs=w_sb["qwc"][:], start=True, stop=True)
            yield
            yqb = sb.tile([P, 4, D], bft, tag="yqb")
            nc.scalar.copy(yqb[:], ps_q[:])
            yield
            r_q, _ = gn_stats(yqb, "q")
            yield
            qn = sb.tile([P, 4, D], bft, tag="qn")
            for c in range(4):
                nc.vector.tensor_scalar(qn[:, c, :], yqb[:, c, :],
                                        r_q[:, c:c + 1], 0.0,
                                        op0=Alu.mult, op1=Alu.max)
            yield
            ps_t = psK.tile([P, 4, D], bft, space="PSUM", tag="k")
            for c in range(4):
                nc.tensor.transpose(ps_t[:, c, :], qn[:, c, :], ident_bf[:])
            yield
            qnT = sb.tile([P, 4, D], bft, tag="qnT")
            nc.vector.tensor_copy(qnT[:], ps_t[:])
            yield
            ps_q2 = psM.tile([P, 4, D], f32, space="PSUM", tag="m")
            for c in range(4):
                nc.tensor.matmul(ps_q2[:, c, :], lhsT=qnT[:, c, :],
                                 rhs=w_sb["Wqc"][:], start=True, stop=True)
            yield
            # ps_q2[p, c, :] = Q2[node g*512+c*128+p] -> Q2sb rank g*4+c ok
            nc.scalar.copy(Q2sb[:, g * 4:(g + 1) * 4, :], ps_q2[:])
            nc.sync.dma_start(
                Gd[g * 512:(g + 1) * 512, :].rearrange(
                    "(p c) f -> p c f", p=P),
                zt[:])
            yield

        def run_interleaved(mk, n, ilv):
            i = 0
            while i < n:
                m = min(ilv, n - i)
                gens = [mk(i + j) for j in range(m)]
                alive = True
                while alive:
                    alive = False
                    for g_ in gens:
                        try:
                            next(g_)
                            alive = True
                        except StopIteration:
                            pass
                i += m

        # ---------- phase 2: edge pipeline ----------
        dt_bufs = {}
        q2_bufs = {}
        q2_enabled = [False]
        lo_bufs = {}
        hi_bufs = {}
        gst_bufs = {}

        def issue_dt(gi):
            nt = min(G_TILES, NT - gi * G_TILES)
            buf = gb.tile([4, G_TILES * P], bft, tag="dt4")
            nc.sync.dma_start(buf[:, :nt * P],
                              t_dT4[:, gi * G_TILES * P:(gi * G_TILES + nt) * P])
            dt_bufs[gi] = buf

        def issue_q2(gi):
            nt = min(G_TILES, NT - gi * G_TILES)
            buf = gb.tile([P, 1, G_TILES * P], bft, tag="q2g")
            nc.gpsimd.dma_gather(
                out_ap=buf[:, :, :nt * P], in_ap=Q2sb[:],
                idxs_ap=qidx[:, gi * G_TILES * 8:(gi * G_TILES + nt) * 8],
                num_idxs=nt * P, num_idxs_reg=nt * P, elem_size=D,
                transpose=True, single_packet=False,
                sbuf_tokens_per_rank=P, sbuf_free_dim_per_rank=D * 2)
            q2_bufs[gi] = buf

        def issue_w(gi, half):
            n_str, src, idxt, bufs, tag = (
                (NLO_T, t_ctx[:CTX_HALF, :], wlo, lo_bufs, "clo") if half == 0
                else (NHI_T, t_ctx[CTX_HALF:, :], whi, hi_bufs, "chi"))
            nt = min(G_TILES, n_str - gi * G_TILES)
            buf = gb.tile([P, 1, G_TILES * P], bft, tag=tag)
            nc.gpsimd.dma_gather(
                out_ap=buf[:, :, :nt * P], in_ap=src,
                idxs_ap=idxt[:, gi * G_TILES * 8:(gi * G_TILES + nt) * 8],
                num_idxs=nt * P, num_idxs_reg=nt * P, elem_size=D,
                transpose=True, single_packet=False)
            bufs[gi] = buf

        def tmeta(t):
            return (0, t) if t < NLO_T else (1, t - NLO_T)

        def ensure_chunk(gi):
            """Issue gather DMAs covering global gather-chunk gi."""
            if gi * G_TILES >= NT:
                return
            if gi not in dt_bufs:
                issue_dt(gi)
            for t in range(gi * G_TILES, min((gi + 1) * G_TILES, NT)):
                half, si = tmeta(t)
                wgi = si // G_TILES
                if half == 0 and wgi not in lo_bufs:
                    issue_w(wgi, 0)
                if half == 1 and wgi not in hi_bufs:
                    issue_w(wgi, 1)
            if q2_enabled[0] and gi not in q2_bufs:
                issue_q2(gi)

        # prefetch the first two gather chunks' dT4 + ctx gathers before
        # phase 1 so the DMA engines fill the pipeline under phase-1 compute
        # (q2 gathers depend on the Q2sb table and must stay after phase 1).
        for _gi in range(3):
            if _gi * G_TILES < NT:
                issue_dt(_gi)
                for _t in range(_gi * G_TILES, min((_gi + 1) * G_TILES, NT)):
                    _half, _si = tmeta(_t)
                    _wgi = _si // G_TILES
                    if _half == 0 and _wgi not in lo_bufs:
                        issue_w(_wgi, 0)
                    if _half == 1 and _wgi not in hi_bufs:
                        issue_w(_wgi, 1)

        # (phase-2 gather helpers defined above, before phase 1)

        def st_stages(s):
            """Pipeline for super-tile s (4 tiles), stage-interleaved."""
            tiles = [4 * s + c for c in range(4)]
            gi0, off0 = divmod(4 * s, G_TILES)
            sc0, soff0 = divmod(4 * s, SC_TILES)
            ensure_chunk(gi0)
            ensure_chunk(gi0 + 1)
            if sc0 not in gst_bufs:
                gst_bufs[sc0] = gp.tile([P, SC_TILES, D], bft, tag="gst",
                                        name="gst")
            yield
            # L1: y1T [f, 512] = w1_aug.T @ dT4
            ps_y1 = psY.tile([P, 4 * D], f32, space="PSUM", tag="y")
            nc.tensor.matmul(ps_y1[:], lhsT=w_sb["w1_aug"][:],
                             rhs=dt_bufs[gi0][:, off0 * P:(off0 + 4) * P],
                             start=True, stop=True)
            yield
            # relu -> r1T bf16 (Act)
            r1T = sb.tile([P, 4 * D], bft, tag="r1T")
            nc.scalar.activation(r1T[:], ps_y1[:], Act.Relu)
            yield
            # L2 (centered) -> ps2 edge-major
            ps2 = psM.tile([P, 4, D], f32, space="PSUM", tag="m")
            for c in range(4):
                nc.tensor.matmul(ps2[:, c, :], lhsT=r1T[:, c * D:(c + 1) * D],
                                 rhs=w_sb["W2c"][:], start=True, stop=True)
            yield
            # y2b copy (Act)
            y2b = sb.tile([P, 4, D], bft, tag="y2b")
            nc.scalar.copy(y2b[:], ps2[:])
            yield
            # GN1 stats (DVE AMR + Act sqrt + DVE recip)
            r1e, _ = gn_stats(y2b, "d")
            yield
            # GN1 apply -> h (DVE)
            h = sb.tile([P, 4, D], bft, tag="h")
            for c in range(4):
                nc.vector.tensor_scalar(h[:, c, :], y2b[:, c, :],
                                        r1e[:, c:c + 1], 0.0,
                                        op0=Alu.mult, op1=Alu.max)
            yield
            # T(h)
            psT = psK.tile([P, 4, D], bft, space="PSUM", tag="k")
            for c in range(4):
                nc.tensor.transpose(psT[:, c, :], h[:, c, :], ident_bf[:])
            yield
            # hT copy split Act/DVE for engine balance
            hT = sb.tile([P, 4, D], bft, tag="hT")
            nc.scalar.copy(hT[:, 0:2, :], psT[:, 0:2, :])
            nc.vector.tensor_copy(hT[:, 2:4, :], psT[:, 2:4, :])
            yield
            # C1 = h@Wd' + ctx[wi]@Wc' + Q2[hi] (identity mm)
            ps3 = psL.tile([P, 4, D], f32, space="PSUM", tag="l")
            for c, t in enumerate(tiles):
                half, si = tmeta(t)
                gi, off = divmod(si, G_TILES)
                cbuf = lo_bufs[gi] if half == 0 else hi_bufs[gi]
                qgi, qoff = divmod(t, G_TILES)
                nc.tensor.matmul(ps3[:, c, :], lhsT=hT[:, c, :],
                                 rhs=w_sb["Wdc"][:], start=True, stop=False)
                nc.tensor.matmul(ps3[:, c, :],
                                 lhsT=cbuf[:, 0, off * P:(off + 1) * P],
                                 rhs=w_sb["Wcc"][:], start=False, stop=False)
                nc.tensor.matmul(ps3[:, c, :],
                                 lhsT=q2_bufs[qgi][:, 0, qoff * P:(qoff + 1) * P],
                                 rhs=ident_bf[:],
                                 start=False, stop=True)
            yield
            # g0b copy (Act)
            g0b = sb.tile([P, 4, D], bft, tag="g0b")
            nc.scalar.copy(g0b[:], ps3[:])
            yield
            # GN2 stats
            r2, _ = gn_stats(g0b, "c")
            yield
            # GN2 apply -> g into scatter staging (DVE)
            gst = gst_bufs[sc0]
            for c in range(4):
                nc.vector.tensor_scalar(gst[:, soff0 + c, :], g0b[:, c, :],
                                        r2[:, c:c + 1], 0.0,
                                        op0=Alu.mult, op1=Alu.max)
            yield
            # scatter-add chunk when staging is full
            if soff0 + 4 == SC_TILES:
                nc.gpsimd.dma_scatter_add(
                    out_ap=Gd[:], in_ap=gst[:, :, :],
                    idxs_ap=sidx[:, sc0 * SC_TILES * 8:(sc0 + 1) * SC_TILES * 8],
                    num_idxs=SC_TILES * P, num_idxs_reg=SC_TILES * P,
                    elem_size=D, single_packet=False)
            yield

        # pad-row zero for scatter TRASH targets: no deps, emit before the
        # merged pool so early scatters don't order against it awkwardly.
        nc.sync.dma_start(Gd[NPC:NPC + P, :], zt[:, 0, :])

        # Merged emission: phase-1 groups + the first EARLY phase-2
        # supertiles. Their stages up to hT depend only on the prefetched
        # dT4/ctx chunks, so they soak up phase-1's idle engine time; their
        # q2-dependent stages simply wait (the q2 descgen head-blocking the
        # Pool queue is harmless here — no other Pool work is pending until
        # after phase 1).
        EARLY = 4
        PRE_Q2_STEPS = 9            # stages 0..8: through the hT copy
        p1g = [p1_stages(g) for g in range(NGRP)]
        eg = [st_stages(s) for s in range(EARLY)]
        done1 = [False] * len(p1g)
        ecnt = [0] * EARLY
        while not all(done1):
            for i, g_ in enumerate(p1g):
                if not done1[i]:
                    try:
                        next(g_)
                    except StopIteration:
                        done1[i] = True
            for j, g_ in enumerate(eg):
                if ecnt[j] < PRE_Q2_STEPS:
                    try:
                        next(g_)
                        ecnt[j] += 1
                    except StopIteration:
                        ecnt[j] = PRE_Q2_STEPS
        # phase 1 fully emitted: q2 gathers may now be issued (they must be
        # emitted after every Q2sb write for correct dependency tracking)
        q2_enabled[0] = True
        ensure_chunk(0)
        ensure_chunk(1)
        alive = True
        while alive:
            alive = False
            for g_ in eg:
                try:
                    next(g_)
                    alive = True
                except StopIteration:
                    pass


        ILV = 8
        grp = 0
        while EARLY + grp * ILV < NST:
            n = min(ILV, NST - EARLY - grp * ILV)
            gens = [st_stages(EARLY + grp * ILV + j) for j in range(n)]
            alive = True
            while alive:
                alive = False
                for g_ in gens:
                    try:
                        next(g_)
                        alive = True
                    except StopIteration:
                        pass
            grp += 1

        # ---------- phase 3: node finale ----------
        # Phase-3 "(p c) f" loads of the permuted Gd/resb rows yield
        # tile[p,c] = node g*512 + c*128 + p — the original mapping, so the
        # A1 term can be computed directly from the matching agtsT columns.
        def p3_stages(g):
            # res load first: it has no scatter dependency, so it streams
            # during the end-of-phase-2 drain instead of queueing behind the
            # scatter-gated Gsb load in SP's in-order queue.
            res_sb = sb.tile([P, 4, D], bft, tag="res_sb")
            nc.sync.dma_start(
                res_sb[:],
                t_resb[g * 512:(g + 1) * 512, :].rearrange(
                    "(p c) f -> p c f", p=P))
            Gsb = sb.tile([P, 4, D], bft, tag="Gsb")
            nc.sync.dma_start(
                Gsb[:],
                Gd[g * 512:(g + 1) * 512, :].rearrange("(p c) f -> p c f", p=P))
            yield
            psTG = psK.tile([P, 4, D], bft, space="PSUM", tag="k")
            for c in range(4):
                nc.tensor.transpose(psTG[:, c, :], Gsb[:, c, :], ident_bf[:])
            yield
            GT = sb.tile([P, 4, D], bft, tag="GT")
            nc.vector.tensor_copy(GT[:], psTG[:])
            yield
            # S^T = ctx_w2'^T G^T + A1^T (feature-major)
            ps_S = psM.tile([P, 4, D], f32, space="PSUM", tag="m")
            for c in range(4):
                nc.tensor.matmul(ps_S[:, c, :], lhsT=w_sb["ctxw2c"][:],
                                 rhs=GT[:, c, :], start=True, stop=False)
                j = g * 4 + c
                nc.tensor.matmul(ps_S[:, c, :], lhsT=w_sb["agtwc"][:],
                                 rhs=agtsT[:, j * P:(j + 1) * P],
                                 start=False, stop=True)
            yield
            o1uT = sb.tile([P, 4, D], bft, tag="o1uT")
            nc.scalar.activation(o1uT[:], ps_S[:], Act.Relu)
            yield
            ps_l = psL.tile([P, 4, D], f32, space="PSUM", tag="l")
            for c in range(4):
                nc.tensor.matmul(ps_l[:, c, :], lhsT=o1uT[:, c, :],
                                 rhs=w_sb["linwc"][:], start=True, stop=True)
            yield
            zb = sb.tile([P, 4, D], bft, tag="zb")
            nc.scalar.copy(zb[:], ps_l[:])
            yield
            r_l, _ = gn_stats(zb, "l")
            yield
            # fin = relu(zb*rinv + res): STT per chunk then one max
            fin = sb.tile([P, 4, D], bft, tag="fin")
            for c in range(4):
                nc.vector.scalar_tensor_tensor(
                    out=fin[:, c, :], in0=zb[:, c, :],
                    scalar=r_l[:, c:c + 1], in1=res_sb[:, c, :],
                    op0=Alu.mult, op1=Alu.add)
            yield
            nc.gpsimd.tensor_scalar_max(fin[:], fin[:], 0.0)
            nc.sync.dma_start(
                t_out[g * 512:(g + 1) * 512, :].rearrange(
                    "(p c) f -> p c f", p=P),
                fin[:])
            yield

        run_interleaved(p3_stages, NGRP, 8)

    nc.compile()
    return nc


_cached = {}
_extra_run_kwargs = {}
_last_results = None


def run_traced(inputs):
    global _extra_run_kwargs
    _extra_run_kwargs = dict(trace=True)
    try:
        kernel(**inputs)
    finally:
        _extra_run_kwargs = {}
    return _last_results


def kernel(agts, ctx, agt_ctrs, ctx_ctrs, hi, wi,
           dist_w1, dist_b1, dist_w2, dist_gw, dist_gb,
           q_w, q_gw, q_gb,
           ctx_w1, ctx_gw, ctx_gb, ctx_w2,
           agt_w, norm_w, norm_b,
           lin_w, lin_gw, lin_gb):
    for name, arr, val in (("dist_gw", dist_gw, 1), ("dist_gb", dist_gb, 0),
                           ("q_gw", q_gw, 1), ("q_gb", q_gb, 0),
                           ("ctx_gw", ctx_gw, 1), ("ctx_gb", ctx_gb, 0),
                           ("norm_w", norm_w, 1), ("norm_b", norm_b, 0),
                           ("lin_gw", lin_gw, 1), ("lin_gb", lin_gb, 0)):
        assert np.allclose(np.asarray(arr), val), f"{name} must be trivial"

    C = np.eye(D, dtype=np.float64) - 1.0 / D   # GN mean-centering projector
    ctx_w1 = np.asarray(ctx_w1, np.float64)
    w1 = np.asarray(dist_w1, np.float32)
    b1 = np.asarray(dist_b1, np.float32)
    w1_aug = np.zeros((4, D), np.float32)
    w1_aug[0:2] = w1
    w1_aug[2] = b1
    weights = dict(
        w1_aug=w1_aug.astype(bf16),
        W2c=(np.asarray(dist_w2, np.float64) @ C).astype(bf16),
        Wdc=(ctx_w1[0:D] @ C).astype(bf16),
        Wqc=(ctx_w1[D:2 * D] @ C).astype(bf16),
        Wcc=(ctx_w1[2 * D:3 * D] @ C).astype(bf16),
        qwc=(np.asarray(q_w, np.float64) @ C).astype(bf16),
        agtwc=(np.asarray(agt_w, np.float64) @ C).astype(bf16),
        ctxw2c=(np.asarray(ctx_w2, np.float64) @ C).astype(bf16),
        linwc=(np.asarray(lin_w, np.float64) @ C).astype(bf16),
    )

    cores, NLO_T, NHI_T = _host_prep(agts, ctx, agt_ctrs, ctx_ctrs, hi, wi)
    key = (NLO_T, NHI_T)
    if key not in _cached:
        _cached[key] = _build_program(NLO_T, NHI_T)
    nc = _cached[key]

    shared = dict(ctx_bf=np.ascontiguousarray(
        np.asarray(ctx, np.float32).astype(bf16)), **weights)
    in_maps = []
    for k in range(NCORES):
        m = dict(cores[k])
        m.update(shared)
        in_maps.append(m)

    res = run_bass_kernel_spmd(nc, in_maps, core_ids=list(range(NCORES)),
                               **_extra_run_kwargs)
    globals()["_last_results"] = res
    sperm = _snode(np.arange(NPC))   # out rows are in storage order
    out = np.concatenate([res.results[k]["out"][sperm] for k in range(NCORES)],
                         axis=0)
    return out.astype(np.float32)


if __name__ == "__main__":
    pass
